# revision 13
# baseline (speedup 1.0000x reference)
"""Trainium2 Bass kernel for nn_Block_34711925686730 (dense_transformer).

Pipeline per image (data-parallel over batch, 4 images / NeuronCore):
  LN(channels) -> iterative KL-NNMF grouped conv (25 iters) -> residual
  -> LN(channels) -> MLP (gelu) -> residual.

NNMF runs in two phases:
  * N8 fp8 iterations: grouped 3x3 convs as fp8e4m3 DoubleRow matmuls
    (2 taps contracted per instruction at 0.5 PE-cycles/row).  Operands
    are power-of-2 pre-scaled (dict x2048/x64, h x128, nu x8) so values
    sit in e4m3's normal range; an extra "eps tap" (0.25 into row 0,
    reading a constant-1.0 h-block) adds the reference's 1e-6 recon
    floor for free and makes the reciprocals NaN-safe.  recon recip
    runs on the Act engine as exp(-ln(x)); nu/u/h updates are
    scalar_tensor_tensor ops split across DVE and GpSimd.
  * NB16 bf16 tail iterations (accurate dict) contract the fp8-phase
    perturbation back toward the f32 fixed point; h is stored x128 in
    both phases so the hand-off is a plain buffer switch.

Layout: channels on partitions (3 blocks of 128 = 2 conv groups of 64),
spatial flattened into a zero-padded 30x30 = 900-col free dim per image.
LN stat sums and fc1 run in fp32r (tf32); fc2 and the tail convs in
bf16.  Channel reductions use ones-matmuls that also broadcast the sum
to all partitions.
"""

import os
import numpy as np

DIM = 384
HEADS = 6
N8 = int(os.environ.get("K_N8", "20"))      # fp8 NNMF iterations
NB16 = int(os.environ.get("K_NB16", "5"))   # bf16 tail iterations
NB = int(os.environ.get("K_NB", "4"))       # images per core
MLP_HID = 4 * DIM
EPS = 1e-6
H = W = 28
NCORES = 8
NBLK = 3          # channel blocks of 128
PW = 30           # padded width
PLEN = 900        # padded spatial length (30*30)
S0 = 31           # first conv-output flat index (y=1,x=1)
R0 = 30           # stats/MLP range start (even, = (1,0))
RL = 840          # stats/MLP columns [30, 870)
NJ = MLP_HID // 128  # 12
N14 = 14 * 28

# fp8 scales (all powers of two; folded exactly through the pipeline)
SWB = 2048.0      # recon dictionary scale
SWF = 64.0        # fwd dictionary scale
SNU = 8.0         # nu storage scale
SH = 128.0        # h storage scale (fp8 phase AND bf16 tail)
SC = SNU * SWB * SH          # 2^21: xnn_s = SC * xnn
C1 = 480.0 / (SWB * SH)      # fp8-phase recip clamp (ratio cap 480)
C3 = 1e6 / SH                # tail recip clamp (reference 1e6 cap)
KU8 = 1.0 / (SWF * SNU)      # fp8-phase u scalar  (2^-9)
KUT = SH / SC                # tail u scalar       (2^-14)
EPSW8 = 0.25                 # fp8 eps tap value -> recon floor ~0.95e-6
EPSWT = SH * 1e-6            # tail eps tap value -> recon floor 1e-6

_cache = {}


def _round_tf32(a):
    bits = np.ascontiguousarray(a, dtype=np.float32).view(np.uint32)
    r = bits + np.uint32(0x0FFF) + ((bits >> np.uint32(13)) & np.uint32(1))
    return (r & np.uint32(0xFFFFE000)).view(np.float32).copy()


def _build():
    import concourse.bacc as bacc
    import concourse.mybir as mybir
    import concourse.tile as tile

    F32 = mybir.dt.float32
    F32R = mybir.dt.float32r
    BF16 = mybir.dt.bfloat16
    FP8 = mybir.dt.float8e4
    F16 = mybir.dt.float16
    AF = mybir.ActivationFunctionType
    op = mybir.AluOpType
    PM = mybir.MatmulPerfMode

    nc = bacc.Bacc("TRN2", target_bir_lowering=False, debug=False)

    x_ext = nc.declare_dram_parameter("x", [NB, DIM, H, W], F32R, isOutput=False)
    afwd_ext = nc.declare_dram_parameter("afwd", [NBLK, 128, 9, 128], BF16, isOutput=False)
    abwd_ext = nc.declare_dram_parameter("abwd", [NBLK, 128, 10, 128], BF16, isOutput=False)
    wf8_ext = nc.declare_dram_parameter("wf8", [NBLK, 128, 10, 128], FP8, isOutput=False)
    wb8_ext = nc.declare_dram_parameter("wb8", [NBLK, 128, 10, 128], FP8, isOutput=False)
    w1_ext = nc.declare_dram_parameter("w1", [NBLK, 128, NJ, 128], F32R, isOutput=False)
    w2_ext = nc.declare_dram_parameter("w2", [NJ, 128, NBLK, 128], BF16, isOutput=False)
    g1_ext = nc.declare_dram_parameter("g1", [NBLK, 128], F32, isOutput=False)
    b1_ext = nc.declare_dram_parameter("b1", [NBLK, 128], F32, isOutput=False)
    g2_ext = nc.declare_dram_parameter("g2", [NBLK, 128], F32, isOutput=False)
    b2_ext = nc.declare_dram_parameter("b2", [NBLK, 128], F32, isOutput=False)
    bf1_ext = nc.declare_dram_parameter("bf1", [NJ, 128], F32, isOutput=False)
    bf2_ext = nc.declare_dram_parameter("bf2", [NBLK, 128], F32, isOutput=False)
    out_ext = nc.declare_dram_parameter("out", [NB, DIM, H, W], F32, isOutput=True)

    # PSUM accumulation groups must not cross a 512-col (2KB) bank boundary
    RCH = [(0, 512), (512, RL - 512)]   # stats/MLP chunks (within [R0, R0+RL))
    HL = [(1, 0), (15, 512)]            # (first interior row, psum col)
    # DR tap pairs: (0,1),(2,3),(4,5),(6,7),(8,aux).  The aux read of the
    # last pair is stride `d8` away; its weights live in tap slot 9.
    PAIR_D = []
    for pair in range(4):
        t0 = 2 * pair
        ky0, kx0 = t0 // 3, t0 % 3
        ky1, kx1 = (t0 + 1) // 3, (t0 + 1) % 3
        PAIR_D.append((ky0, kx0, (ky1 - ky0) * PW + (kx1 - kx0)))

    with tile.TileContext(nc) as tc:
        with (
            tc.tile_pool(name="singles", bufs=1) as singles,
            tc.tile_pool(name="img", bufs=1) as pimg,
            tc.tile_pool(name="stats", bufs=3) as stats,
            tc.tile_pool(name="psA", bufs=3, space="PSUM") as psA,
            tc.tile_pool(name="psB", bufs=1, space="PSUM") as psB,
        ):
            # ---- weights / params resident in SBUF ----
            onesf = singles.tile([128, 128], F32)
            nc.vector.memset(onesf, 1.0)
            ones_r = singles.tile([128, 128], F32R)
            nc.vector.tensor_copy(ones_r, onesf)
            ones_b = singles.tile([128, 128], BF16)
            nc.vector.memset(ones_b, 1.0)
            ones_sc = singles.tile([128, 128], BF16, name="ones_sc", tag="ones_sc")
            nc.vector.memset(ones_sc, 1.0 / SC)
            ones_cs = singles.tile([128, 128], BF16, name="ones_cs", tag="ones_cs")
            nc.vector.memset(ones_cs, 1.0 / SH)
            wfwd = []
            wbwd = []
            wf8 = []
            wb8 = []
            for b in range(NBLK):
                wf = singles.tile([128, 9, 128], BF16, name=f"wfwd{b}", tag=f"wfwd{b}")
                nc.sync.dma_start(out=wf, in_=afwd_ext[b])
                wfwd.append(wf)
                wb = singles.tile([128, 10, 128], BF16, name=f"wbwd{b}", tag=f"wbwd{b}")
                nc.sync.dma_start(out=wb, in_=abwd_ext[b])
                wbwd.append(wb)
                t8 = singles.tile([128, 10, 128], FP8, name=f"wf8{b}", tag=f"wf8{b}")
                nc.sync.dma_start(out=t8, in_=wf8_ext[b])
                wf8.append(t8)
                t8 = singles.tile([128, 10, 128], FP8, name=f"wb8{b}", tag=f"wb8{b}")
                nc.sync.dma_start(out=t8, in_=wb8_ext[b])
                wb8.append(t8)
            w1t = []
            for kb in range(NBLK):
                t = singles.tile([128, NJ, 128], F32R, name=f"w1t{kb}", tag=f"w1t{kb}")
                nc.sync.dma_start(out=t, in_=w1_ext[kb])
                w1t.append(t)
            w2t = []
            for jb in range(NJ):
                t = singles.tile([128, NBLK, 128], BF16, name=f"w2t{jb}", tag=f"w2t{jb}")
                nc.sync.dma_start(out=t, in_=w2_ext[jb])
                w2t.append(t)

            def load_param(ext, n, name):
                t = singles.tile([128, n], F32, name=name, tag=name)
                nc.sync.dma_start(out=t, in_=ext[:, :].rearrange("b p -> p b"))
                return t

            eps1_t = singles.tile([128, 1], F32, name="eps1_t", tag="eps1_t")
            nc.vector.memset(eps1_t, EPS)
            eps2_t = singles.tile([128, 1], F32, name="eps2_t", tag="eps2_t")
            nc.vector.memset(eps2_t, 1e-5)

            g1t = load_param(g1_ext, NBLK, "g1t")
            b1t = load_param(b1_ext, NBLK, "b1t")
            g2t = load_param(g2_ext, NBLK, "g2t")
            b2t = load_param(b2_ext, NBLK, "b2t")
            bf1t = load_param(bf1_ext, NJ, "bf1t")
            bf2t = load_param(bf2_ext, NBLK, "bf2t")

            def pad3(t, b):
                # [128, 30, 30] view of block b
                return t[:, b, :].rearrange("p (r c) -> p r c", c=PW)

            def i4(t, b):
                return (pad3(t, b)[:, 1:29, 1:29]
                        .rearrange("p (two r) c -> p two r c", two=2))

            def i4all(t):
                # [128, 3, 28, 28] interior view across blocks 0-2
                return (t.rearrange("p b (r c) -> p b r c", c=PW)
                        [:, 0:3, 1:29, 1:29])

            def i4allx(t):
                # same, but F32-typed view of an f32r tile
                return (t[:, :, :].bitcast(F32)
                        .rearrange("p b (r c) -> p b r c", c=PW)
                        [:, 0:3, 1:29, 1:29])

            def i4x(t, b):
                return (t[:, b, :].bitcast(F32)
                        .rearrange("p (r c) -> p r c", c=PW)[:, 1:29, 1:29]
                        .rearrange("p (two r) c -> p two r c", two=2))

            def ps4(ps):
                return (ps.rearrange("p (two x) -> p two x", two=2)
                        [:, :, 0:N14]
                        .rearrange("p two (r c) -> p two r c", c=28))

            def ps2(ps):
                return (ps.rearrange("p (two x) -> p two x", two=2)
                        [:, :, 0:N14])

            def layernorm(src, dst_slice_fn, eps, gt, bt, post):
                """Channel LN over the 3 partition blocks of `src`
                [128,NBLK,PLEN] (f32r AP) on range [R0, R0+RL)."""
                s1 = psA.tile([128, 1024], F32, tag="conv")
                for (c0, cn) in RCH:
                    for b in range(NBLK):
                        nc.tensor.matmul(
                            out=s1[:, c0:c0 + cn],
                            lhsT=ones_r,
                            rhs=src[:, b, R0 + c0: R0 + c0 + cn],
                            start=(b == 0),
                            stop=(b == NBLK - 1),
                        )
                s2 = psA.tile([128, 1024], F32, tag="conv")
                for b in range(NBLK):
                    sq = stats.tile([128, RL], BF16, tag="sq", bufs=2)
                    nc.scalar.activation(
                        out=sq, in_=src[:, b, R0:R0 + RL].bitcast(F32), func=AF.Square
                    )
                    for (c0, cn) in RCH:
                        nc.tensor.matmul(
                            out=s2[:, c0:c0 + cn],
                            lhsT=ones_b,
                            rhs=sq[:, c0:c0 + cn],
                            start=(b == 0),
                            stop=(b == NBLK - 1),
                        )
                m = stats.tile([128, RL], F32, tag="mstat", bufs=4)
                nc.vector.tensor_scalar_mul(m, s1[:, 0:RL], 1.0 / DIM)
                t2 = stats.tile([128, RL], F32, tag="mstat", bufs=4)
                nc.vector.tensor_scalar_mul(t2, s2[:, 0:RL], 1.0 / DIM)
                msq = stats.tile([128, RL], F32, tag="mstat", bufs=4)
                nc.scalar.activation(out=msq, in_=m, func=AF.Square)
                v = stats.tile([128, RL], F32, tag="mstat", bufs=4)
                nc.vector.tensor_sub(v, t2, msq)
                sd = stats.tile([128, RL], F32, tag="mstat", bufs=4)
                nc.scalar.activation(out=sd, in_=v, func=AF.Sqrt, bias=eps)
                rstd = stats.tile([128, RL], F32, tag="mstat", bufs=4)
                nc.vector.reciprocal_approx_fast(out=rstd, in_=sd)

                if post == "ln1":
                    z0s = []
                    for b in range(NBLK):
                        d = stats.tile([128, RL], F32, tag="dtmp", bufs=2)
                        nc.vector.tensor_sub(d, src[:, b, R0:R0 + RL].bitcast(F32), m)
                        xn = stats.tile([128, RL], F32, tag="dtmp2", bufs=1)
                        nc.vector.tensor_mul(xn, d, rstd)
                        aff = stats.tile([128, RL], F32, tag="dtmp", bufs=2)
                        nc.vector.tensor_scalar(
                            aff, xn, gt[:, b:b + 1], bt[:, b:b + 1], op.mult, op.add
                        )
                        z0 = stats.tile([128, RL], BF16, tag="z0", bufs=3)
                        nc.vector.tensor_scalar_max(z0, aff, EPS)
                        z0s.append(z0)
                    s0ps = psB.tile([128, 1024], F32, tag="sum")
                    for (c0, cn) in RCH:
                        for b in range(NBLK):
                            nc.tensor.matmul(
                                out=s0ps[:, c0:c0 + cn],
                                lhsT=ones_sc,
                                rhs=z0s[b][:, c0:c0 + cn],
                                start=(b == 0),
                                stop=(b == NBLK - 1),
                            )
                    rs = stats.tile([128, RL], F32, tag="rcpln", bufs=1)
                    nc.vector.reciprocal_approx_fast(out=rs, in_=s0ps[:, 0:RL])
                    for b in range(NBLK):
                        nc.vector.scalar_tensor_tensor(
                            out=dst_slice_fn(b), in0=rs, scalar=1e6 * SC,
                            in1=z0s[b], op0=op.min, op1=op.mult,
                        )
                else:
                    for b in range(NBLK):
                        d = stats.tile([128, RL], F32, tag="dtmp", bufs=2)
                        nc.vector.tensor_sub(d, src[:, b, R0:R0 + RL].bitcast(F32), m)
                        xn = stats.tile([128, RL], F32, tag="dtmp2", bufs=1)
                        nc.vector.tensor_mul(xn, d, rstd)
                        nc.vector.tensor_scalar(
                            dst_slice_fn(b), xn, gt[:, b:b + 1], bt[:, b:b + 1],
                            op.mult, op.add,
                        )

            # ================= per image (pairs interleaved) =================
            def setup_image(img):
                xpad = pimg.tile([128, NBLK, PLEN], F32R, tag="xpad", bufs=2,
                                 name=f"xpad{img}")
                xnn = pimg.tile([128, NBLK, PLEN], F32R, tag="xnn", bufs=2,
                                name=f"xnn{img}")
                hT = pimg.tile([128, 4, PLEN], FP8, tag="h8", bufs=2,
                               name=f"h8{img}")
                nuT = pimg.tile([128, 4, PLEN], FP8, tag="nu8", bufs=2,
                                name=f"nu8{img}")
                hB = pimg.tile([128, 4, PLEN], BF16, tag="hb", bufs=2,
                               name=f"hb{img}")
                uB = pimg.tile([128, 4, PLEN], BF16, tag="ub", bufs=2,
                               name=f"ub{img}")
                nc.gpsimd.memset(hT, 0.0)
                nc.gpsimd.memset(nuT, 0.0)
                nc.gpsimd.memset(hB, 0.0)
                nc.gpsimd.memset(uB, 0.0)
                nc.gpsimd.memset(hT[:, 3, :], 1.0)
                nc.gpsimd.memset(hB[:, 3, :], 1.0)
                nc.gpsimd.memset(uB[:, 3, :], 5.12e-4)
                for b in range(NBLK):
                    nc.sync.dma_start(
                        out=pad3(xpad, b)[:, 1:29, 1:29],
                        in_=x_ext[img, b * 128:(b + 1) * 128, :, :],
                    )
                    # h init: h = 1/DIM -> h_s = SH/DIM = 1/3
                    nc.gpsimd.memset(pad3(hT, b)[:, 1:29, 1:29], 1.0 / NBLK)
                layernorm(
                    xpad, lambda b: xnn[:, b, R0:R0 + RL], eps1_t, g1t, b1t, "ln1"
                )
                return xpad, xnn, hT, nuT, hB, uB

            def conv_dr(dst_ps, wtile, srcbig, b, aux_delta):
                """3x3 grouped conv on fp8 `srcbig` [128, 4*30, 30] view via 5
                DoubleRow matmuls per half; pair 4 = (tap8, aux at
                +aux_delta with tap-9 weights)."""
                for (r0, c0) in HL:
                    base = b * PW + r0
                    for pair in range(5):
                        if pair < 4:
                            ky0, kx0, delta = PAIR_D[pair]
                        else:
                            ky0, kx0, delta = 2, 2, aux_delta
                        rhs = srcbig[:, base + ky0 - 1: base + ky0 + 13,
                                     kx0: kx0 + 28].unsqueeze(1)
                        rhs.ap[1] = [delta, 2]
                        nc.tensor.matmul(
                            out=dst_ps[:, c0:c0 + N14],
                            lhsT=wtile[:, 2 * pair: 2 * pair + 2, :],
                            rhs=rhs,
                            start=(pair == 0),
                            stop=(pair == 4),
                            perf_mode=PM.DoubleRow,
                        )

            def iter8(ts, last):
                xpad, xnn, hT, nuT, hB, uB = ts
                hbig = hT.rearrange("p b (r c) -> p (b r) c", c=PW)
                nbig = nuT.rearrange("p b (r c) -> p (b r) c", c=PW)
                # recon = conv(h, wt) + eps ; rcp = exp(-ln(recon_ps)) on Act
                lnt = stats.tile([128, NBLK, 784], F16, tag="lnt", bufs=1)
                for b in range(NBLK):
                    ps = psA.tile([128, 1024], F32, tag="conv")
                    conv_dr(ps, wb8[b], hbig, b, (3 - b) * PLEN)
                    nc.scalar.activation(
                        out=lnt[:, b, :].rearrange("p (two x) -> p two x", two=2),
                        in_=ps2(ps), func=AF.Ln,
                    )
                rcp = stats.tile([128, NBLK, 784], BF16, tag="rcp", bufs=2)
                nc.scalar.activation(
                    out=rcp.rearrange("p b x -> p (b x)"),
                    in_=lnt.rearrange("p b x -> p (b x)"),
                    func=AF.Exp, scale=-1.0,
                )
                # nu_s = min(rcp, C1) * xnn_s   (per-block stt, fp8 out)
                for b in range(NBLK):
                    rcp4b = rcp[:, b, :].rearrange(
                        "p (two r c) -> p two r c", two=2, c=28)
                    nc.vector.scalar_tensor_tensor(
                        out=i4(nuT, b), in0=rcp4b, scalar=C1,
                        in1=i4x(xnn, b), op0=op.min, op1=op.mult,
                    )
                # u_b = fwd_ps . h_s = 65536*u  -> bf16 (scale cancels later)
                for b in range(NBLK):
                    ps = psA.tile([128, 1024], F32, tag="conv")
                    conv_dr(ps, wf8[b], nbig, b, PLEN)
                    nc.vector.tensor_mul(i4(uB, b), ps4(ps), i4(hT, b))
                # colsum over 4 blocks (block 3 = 1e-6 eps) ; sinv = 1/cs
                ss = psB.tile([128, 1024], F32, tag="sum")
                for (r0, c0) in HL:
                    for b in range(4):
                        nc.tensor.matmul(
                            out=ss[:, c0:c0 + N14],
                            lhsT=ones_cs,
                            rhs=pad3(uB, b)[:, r0:r0 + 14, 1:29],
                            start=(b == 0),
                            stop=(b == 3),
                        )
                lnt2 = stats.tile([128, 784], F16, tag="lnt2", bufs=1)
                nc.scalar.activation(
                    out=lnt2.rearrange("p (two x) -> p two x", two=2),
                    in_=ps2(ss), func=AF.Ln,
                )
                sinv = stats.tile([128, 784], BF16, tag="sinv", bufs=1)
                nc.scalar.activation(out=sinv, in_=lnt2, func=AF.Exp, scale=-1.0)
                sinv4 = sinv.rearrange("p (two r c) -> p two r c", two=2, c=28)
                hdst = hB if last else hT
                for b in range(NBLK):
                    nc.gpsimd.tensor_mul(i4(hdst, b), sinv4, i4(uB, b))

            def iter16(ts):
                xpad, xnn, hT, nuT, hB, uB = ts
                # recon (bf16, 9 taps + eps tap reading hB block 3)
                for b in range(NBLK):
                    h3 = pad3(hB, b)
                    ps = psA.tile([128, 1024], F32, tag="conv")
                    for (r0, c0) in HL:
                        for t in range(9):
                            ky, kx = t // 3, t % 3
                            nc.tensor.matmul(
                                out=ps[:, c0:c0 + N14],
                                lhsT=wbwd[b][:, t, :],
                                rhs=h3[:, r0 + ky - 1: r0 + ky + 13,
                                       kx: kx + 28],
                                start=(t == 0),
                                stop=False,
                            )
                        nc.tensor.matmul(
                            out=ps[:, c0:c0 + N14],
                            lhsT=wbwd[b][:, 9, :],
                            rhs=pad3(hB, 3)[:, r0:r0 + 14, 1:29],
                            start=False,
                            stop=True,
                        )
                    rcpb = stats.tile([128, RL], F32, tag="mstat", bufs=4)
                    nc.vector.reciprocal_approx_fast(
                        out=rcpb[:, 0:784].rearrange("p (two x) -> p two x", two=2),
                        in_=ps2(ps),
                    )
                    # nu_t = rcp * xnn_s -> bf16 (1e6 cap automatic via eps tap)
                    nc.gpsimd.tensor_mul(
                        i4(uB, b),
                        rcpb[:, 0:784].rearrange(
                            "p (two r c) -> p two r c", two=2, c=28),
                        i4x(xnn, b),
                    )
                # u_b = (fwd_ps * KUT) . hB
                for b in range(NBLK):
                    nu3 = pad3(uB, b)
                    ps = psA.tile([128, 1024], F32, tag="conv")
                    for (r0, c0) in HL:
                        for t in range(9):
                            ky, kx = t // 3, t % 3
                            nc.tensor.matmul(
                                out=ps[:, c0:c0 + N14],
                                lhsT=wfwd[b][:, t, :],
                                rhs=nu3[:, r0 + ky - 1: r0 + ky + 13,
                                        kx: kx + 28],
                                start=(t == 0),
                                stop=(t == 8),
                            )
                    nc.vector.tensor_mul(i4(uB, b), ps4(ps), i4(hB, b))
                # h = u / colsum  (x SH)
                ss = psB.tile([128, 1024], F32, tag="sum")
                for (r0, c0) in HL:
                    for b in range(4):
                        nc.tensor.matmul(
                            out=ss[:, c0:c0 + N14],
                            lhsT=ones_cs,
                            rhs=pad3(uB, b)[:, r0:r0 + 14, 1:29],
                            start=(b == 0),
                            stop=(b == 3),
                        )
                sinvt = stats.tile([128, RL], F32, tag="mstat", bufs=4)
                nc.vector.reciprocal_approx_fast(
                    out=sinvt[:, 0:784].rearrange("p (two x) -> p two x", two=2),
                    in_=ps2(ss),
                )
                sinv4 = sinvt[:, 0:784].rearrange(
                    "p (two r c) -> p two r c", two=2, c=28)
                for b in range(NBLK):
                    nc.gpsimd.tensor_mul(i4(hB, b), sinv4, i4(uB, b))

            def tail_ln(img, ts):
                xpad, xnn, hT, nuT, hB, uB = ts
                # residual: x2 = x + h  (h stored x SH) -> into xnn slot
                for b in range(NBLK):
                    nc.vector.scalar_tensor_tensor(
                        out=xnn[:, b, R0:R0 + RL],
                        in0=hB[:, b, R0:R0 + RL], scalar=1.0 / SH,
                        in1=xpad[:, b, R0:R0 + RL].bitcast(F32),
                        op0=op.mult, op1=op.add,
                    )
                # LN2 -> xn2 (into xpad slot)
                layernorm(
                    xnn,
                    lambda b: xpad[:, b, R0:R0 + RL],
                    eps2_t, g2t, b2t, "ln2",
                )

            def tail_mlp(img, ts):
                xpad, xnn, hT, nuT, hB, uB = ts
                hid = pimg.tile([128, NJ, RL], BF16, tag="hid", bufs=1,
                                name=f"hid{img}")
                for j in range(NJ):
                    hp = psA.tile([128, 1024], F32, tag="conv")
                    for (c0, cn) in RCH:
                        for kb in range(NBLK):
                            nc.tensor.matmul(
                                out=hp[:, c0:c0 + cn],
                                lhsT=w1t[kb][:, j, :],
                                rhs=xpad[:, kb, R0 + c0: R0 + c0 + cn],
                                start=(kb == 0),
                                stop=(kb == NBLK - 1),
                            )
                    nc.scalar.activation(
                        out=hid[:, j, :], in_=hp[:, 0:RL], func=AF.Gelu,
                        bias=bf1t[:, j:j + 1], scale=1.0,
                    )
                for cb in range(NBLK):
                    ops_ = psB.tile([128, 1024], F32, tag="sum")
                    for (c0, cn) in RCH:
                        for j in range(NJ):
                            nc.tensor.matmul(
                                out=ops_[:, c0:c0 + cn],
                                lhsT=w2t[j][:, cb, :],
                                rhs=hid[:, j, c0:c0 + cn],
                                start=(j == 0),
                                stop=(j == NJ - 1),
                            )
                    nc.vector.scalar_tensor_tensor(
                        out=xpad[:, cb, R0:R0 + RL],
                        in0=ops_[:, 0:RL],
                        scalar=bf2t[:, cb:cb + 1],
                        in1=xnn[:, cb, R0:R0 + RL].bitcast(F32),
                        op0=op.add, op1=op.add,
                    )
                for b in range(NBLK):
                    src = (xpad[:, b, :].bitcast(F32)
                           .rearrange("p (r c) -> p r c", c=PW)[:, 1:29, 1:29])
                    nc.sync.dma_start(
                        out=out_ext[img, b * 128:(b + 1) * 128, :, :],
                        in_=src,
                    )

            for p0 in range(0, NB, 2):
                imgs = list(range(p0, min(p0 + 2, NB)))
                tsets = {img: setup_image(img) for img in imgs}
                for it in range(N8):
                    for img in imgs:
                        iter8(tsets[img], last=(it == N8 - 1))
                for it in range(NB16):
                    for img in imgs:
                        iter16(tsets[img])
                for img in imgs:
                    tail_ln(img, tsets[img])
                for img in imgs:
                    tail_mlp(img, tsets[img])

    nc.compile()
    return nc


def _prep_weights(Wc, g1, b1, g2, b2, w_fc1, b_fc1, w_fc2, b_fc2):
    import ml_dtypes

    wp = np.abs(np.asarray(Wc, np.float32))
    wp = wp / np.maximum(wp.sum(axis=(1, 2, 3), keepdims=True), EPS)
    wp4 = wp.reshape(NBLK, 2, 64, 64, 3, 3)  # [b, gi, co, ci, ky, kx]
    afwd = np.zeros((NBLK, 128, 9, 128), np.float32)
    abwd = np.zeros((NBLK, 128, 10, 128), np.float32)
    for b in range(NBLK):
        for gi in range(2):
            blk = wp4[b, gi]
            afwd[b, gi * 64:(gi + 1) * 64, :, gi * 64:(gi + 1) * 64] = (
                blk.transpose(1, 2, 3, 0).reshape(64, 9, 64)
            )
            abwd[b, gi * 64:(gi + 1) * 64, 0:9, gi * 64:(gi + 1) * 64] = (
                blk[:, :, ::-1, ::-1].transpose(0, 2, 3, 1).reshape(64, 9, 64)
            )
    abwd[:, 0, 9, :] = EPSWT
    wf8 = np.zeros((NBLK, 128, 10, 128), np.float32)
    wf8[:, :, 0:9, :] = afwd * SWF
    wb8 = abwd * SWB
    wb8[:, :, 9, :] = 0.0
    wb8[:, 0, 9, :] = EPSW8
    wf8 = wf8.astype(ml_dtypes.float8_e4m3)
    wb8 = wb8.astype(ml_dtypes.float8_e4m3)
    assert np.isfinite(wf8.astype(np.float32)).all()
    assert np.isfinite(wb8.astype(np.float32)).all()
    w1 = np.asarray(w_fc1, np.float32).reshape(NBLK, 128, NJ, 128)
    w2 = np.asarray(w_fc2, np.float32).reshape(NJ, 128, NBLK, 128).astype(
        ml_dtypes.bfloat16
    )
    return {
        "afwd": (afwd / 32.0).astype(ml_dtypes.bfloat16),
        "abwd": abwd.astype(ml_dtypes.bfloat16),
        "wf8": wf8,
        "wb8": wb8,
        "w1": _round_tf32(w1),
        "w2": w2,
        "g1": np.asarray(g1, np.float32).reshape(NBLK, 128),
        "b1": np.asarray(b1, np.float32).reshape(NBLK, 128),
        "g2": np.asarray(g2, np.float32).reshape(NBLK, 128),
        "b2": np.asarray(b2, np.float32).reshape(NBLK, 128),
        "bf1": np.asarray(b_fc1, np.float32).reshape(NJ, 128),
        "bf2": np.asarray(b_fc2, np.float32).reshape(NBLK, 128),
    }


_last_result = None


def kernel(x, g1, b1, Wc, g2, b2, w_fc1, b_fc1, w_fc2, b_fc2):
    global _last_result
    if os.environ.get("JAX_PLATFORMS", "").strip().lower() == "cpu":
        del os.environ["JAX_PLATFORMS"]
    from concourse.bass_utils import run_bass_kernel_spmd

    if "nc" not in _cache:
        _cache["nc"] = _build()
    nc = _cache["nc"]

    shared = _prep_weights(Wc, g1, b1, g2, b2, w_fc1, b_fc1, w_fc2, b_fc2)
    x = np.asarray(x, np.float32)
    assert x.shape == (NB * NCORES, DIM, H, W), x.shape
    in_maps = []
    for c in range(NCORES):
        m = dict(shared)
        m["x"] = np.ascontiguousarray(x[c * NB:(c + 1) * NB])
        in_maps.append(m)

    r = run_bass_kernel_spmd(
        nc, in_maps, list(range(NCORES)),
        trace=bool(os.environ.get("K_TRACE")),
    )
    _last_result = r
    out = np.concatenate(
        [r.results[c]["out"] for c in range(NCORES)], axis=0
    ).astype(np.float32)
    return out


# revision 16
# speedup vs baseline: 1.4671x; 1.4671x over previous
"""Trainium2 Bass kernel for nn_Block_34711925686730 (dense_transformer).

Pipeline per image (data-parallel over batch, 4 images / NeuronCore):
  LN(channels) -> iterative KL-NNMF grouped conv (25 iters) -> residual
  -> LN(channels) -> MLP (gelu) -> residual.

NNMF runs in two phases:
  * N8 fp8 iterations: grouped 3x3 convs as fp8e4m3 DoubleRow matmuls
    (2 taps contracted per instruction at 0.5 PE-cycles/row).  Operands
    are power-of-2 pre-scaled (dict x2048/x64, h x128, nu x8) so values
    sit in e4m3's normal range; an extra "eps tap" (0.25 into row 0,
    reading a constant-1.0 h-block) adds the reference's 1e-6 recon
    floor for free and makes the reciprocals NaN-safe.  recon recip
    runs on the Act engine as exp(-ln(x)); nu/u/h updates are
    scalar_tensor_tensor ops split across DVE and GpSimd.
  * NB16 bf16 tail iterations (accurate dict) contract the fp8-phase
    perturbation back toward the f32 fixed point; h is stored x128 in
    both phases so the hand-off is a plain buffer switch.

Layout: channels on partitions (3 blocks of 128 = 2 conv groups of 64),
spatial flattened into a zero-padded 30x30 = 900-col free dim per image.
LN stat sums and fc1 run in fp32r (tf32); fc2 and the tail convs in
bf16.  Channel reductions use ones-matmuls that also broadcast the sum
to all partitions.
"""

import os
import numpy as np

DIM = 384
HEADS = 6
N8 = int(os.environ.get("K_N8", "20"))      # fp8 NNMF iterations
NB16 = int(os.environ.get("K_NB16", "5"))   # bf16 tail iterations
NB = int(os.environ.get("K_NB", "4"))       # images per core
MLP_HID = 4 * DIM
EPS = 1e-6
H = W = 28
NCORES = 8
NBLK = 3          # channel blocks of 128
PW = 30           # padded width
PLEN = 900        # padded spatial length (30*30)
S0 = 31           # first conv-output flat index (y=1,x=1)
R0 = 30           # stats/MLP range start (even, = (1,0))
RL = 840          # stats/MLP columns [30, 870)
NJ = MLP_HID // 128  # 12
N14 = 14 * 28

# fp8 scales (all powers of two; folded exactly through the pipeline)
SWB = 2048.0      # recon dictionary scale
SWF = 64.0        # fwd dictionary scale
SNU = 8.0         # nu storage scale
SH = 128.0        # h storage scale (fp8 phase AND bf16 tail)
SC = SNU * SWB * SH          # 2^21: xnn_s = SC * xnn
C1 = 480.0 / (SWB * SH)      # fp8-phase recip clamp (ratio cap 480)
C3 = 1e6 / SH                # tail recip clamp (reference 1e6 cap)
KU8 = 1.0 / (SWF * SNU)      # fp8-phase u scalar  (2^-9)
KUT = SH / SC                # tail u scalar       (2^-14)
EPSW8 = 0.25                 # fp8 eps tap value -> recon floor ~0.95e-6
EPSWT = SH * 1e-6            # tail eps tap value -> recon floor 1e-6

_cache = {}


def _round_tf32(a):
    bits = np.ascontiguousarray(a, dtype=np.float32).view(np.uint32)
    r = bits + np.uint32(0x0FFF) + ((bits >> np.uint32(13)) & np.uint32(1))
    return (r & np.uint32(0xFFFFE000)).view(np.float32).copy()


def _build():
    import concourse.bacc as bacc
    import concourse.mybir as mybir
    import concourse.tile as tile
    from concourse.hw_specs import get_activation_tables

    # The act-table chooser is first-match over act_info.json order, which
    # ping-pongs 1283ns table loads between 'natural_log' and
    # 'exp_and_others' for our exp(-ln(x)) reciprocals.  Neuter those two
    # single-function tables (ids keep their positions) so both Ln and Exp
    # resolve to 'natural_log_exp_and_others'.
    _orig_tables = bacc.get_activation_tables

    def _patched_tables(arch):
        tabs = dict(get_activation_tables(arch))
        if "natural_log_exp_and_others" in tabs:
            for name in ("exp_and_others", "natural_log"):
                if name in tabs:
                    tabs[name] = set()
        return tabs

    bacc.get_activation_tables = _patched_tables

    F32 = mybir.dt.float32
    F32R = mybir.dt.float32r
    BF16 = mybir.dt.bfloat16
    FP8 = mybir.dt.float8e4
    F16 = mybir.dt.float16
    AF = mybir.ActivationFunctionType
    op = mybir.AluOpType
    PM = mybir.MatmulPerfMode

    nc = bacc.Bacc("TRN2", target_bir_lowering=False, debug=False)

    x_ext = nc.declare_dram_parameter("x", [NB, DIM, H, W], F32R, isOutput=False)
    afwd_ext = nc.declare_dram_parameter("afwd", [NBLK, 128, 9, 128], BF16, isOutput=False)
    abwd_ext = nc.declare_dram_parameter("abwd", [NBLK, 128, 10, 128], BF16, isOutput=False)
    wf8_ext = nc.declare_dram_parameter("wf8", [NBLK, 128, 10, 128], FP8, isOutput=False)
    wb8_ext = nc.declare_dram_parameter("wb8", [NBLK, 128, 10, 128], FP8, isOutput=False)
    w1_ext = nc.declare_dram_parameter("w1", [NBLK, 128, NJ, 128], F32R, isOutput=False)
    w2_ext = nc.declare_dram_parameter("w2", [NJ, 128, NBLK, 128], BF16, isOutput=False)
    g1_ext = nc.declare_dram_parameter("g1", [NBLK, 128], F32, isOutput=False)
    b1_ext = nc.declare_dram_parameter("b1", [NBLK, 128], F32, isOutput=False)
    g2_ext = nc.declare_dram_parameter("g2", [NBLK, 128], F32, isOutput=False)
    b2_ext = nc.declare_dram_parameter("b2", [NBLK, 128], F32, isOutput=False)
    bf1_ext = nc.declare_dram_parameter("bf1", [NJ, 128], F32, isOutput=False)
    bf2_ext = nc.declare_dram_parameter("bf2", [NBLK, 128], F32, isOutput=False)
    out_ext = nc.declare_dram_parameter("out", [NB, DIM, H, W], F32, isOutput=True)

    # PSUM accumulation groups must not cross a 512-col (2KB) bank boundary
    RCH = [(0, 512), (512, RL - 512)]   # stats/MLP chunks (within [R0, R0+RL))
    HL = [(1, 0), (15, 512)]            # (first interior row, psum col)
    # DR tap pairs: (0,1),(2,3),(4,5),(6,7),(8,aux).  The aux read of the
    # last pair is stride `d8` away; its weights live in tap slot 9.
    PAIR_D = []
    for pair in range(4):
        t0 = 2 * pair
        ky0, kx0 = t0 // 3, t0 % 3
        ky1, kx1 = (t0 + 1) // 3, (t0 + 1) % 3
        PAIR_D.append((ky0, kx0, (ky1 - ky0) * PW + (kx1 - kx0)))

    with tile.TileContext(nc) as tc:
        with (
            tc.tile_pool(name="singles", bufs=1) as singles,
            tc.tile_pool(name="img", bufs=1) as pimg,
            tc.tile_pool(name="stats", bufs=3) as stats,
            tc.tile_pool(name="psA", bufs=3, space="PSUM") as psA,
            tc.tile_pool(name="psB", bufs=1, space="PSUM") as psB,
        ):
            # ---- weights / params resident in SBUF ----
            onesf = singles.tile([128, 128], F32)
            nc.vector.memset(onesf, 1.0)
            ones_r = singles.tile([128, 128], F32R)
            nc.vector.tensor_copy(ones_r, onesf)
            ones_b = singles.tile([128, 128], BF16)
            nc.vector.memset(ones_b, 1.0)
            ones_sc = singles.tile([128, 128], BF16, name="ones_sc", tag="ones_sc")
            nc.vector.memset(ones_sc, 1.0 / SC)
            ones_cs = singles.tile([128, 128], BF16, name="ones_cs", tag="ones_cs")
            nc.vector.memset(ones_cs, 1.0 / SH)
            wfwd = []
            wbwd = []
            wf8 = []
            wb8 = []
            for b in range(NBLK):
                wf = singles.tile([128, 9, 128], BF16, name=f"wfwd{b}", tag=f"wfwd{b}")
                nc.sync.dma_start(out=wf, in_=afwd_ext[b])
                wfwd.append(wf)
                wb = singles.tile([128, 10, 128], BF16, name=f"wbwd{b}", tag=f"wbwd{b}")
                nc.sync.dma_start(out=wb, in_=abwd_ext[b])
                wbwd.append(wb)
                t8 = singles.tile([128, 10, 128], FP8, name=f"wf8{b}", tag=f"wf8{b}")
                nc.sync.dma_start(out=t8, in_=wf8_ext[b])
                wf8.append(t8)
                t8 = singles.tile([128, 10, 128], FP8, name=f"wb8{b}", tag=f"wb8{b}")
                nc.sync.dma_start(out=t8, in_=wb8_ext[b])
                wb8.append(t8)
            w1t = []
            for kb in range(NBLK):
                t = singles.tile([128, NJ, 128], F32R, name=f"w1t{kb}", tag=f"w1t{kb}")
                nc.sync.dma_start(out=t, in_=w1_ext[kb])
                w1t.append(t)
            w2t = []
            for jb in range(NJ):
                t = singles.tile([128, NBLK, 128], BF16, name=f"w2t{jb}", tag=f"w2t{jb}")
                nc.sync.dma_start(out=t, in_=w2_ext[jb])
                w2t.append(t)

            def load_param(ext, n, name):
                t = singles.tile([128, n], F32, name=name, tag=name)
                nc.sync.dma_start(out=t, in_=ext[:, :].rearrange("b p -> p b"))
                return t

            eps1_t = singles.tile([128, 1], F32, name="eps1_t", tag="eps1_t")
            nc.vector.memset(eps1_t, EPS)
            eps2_t = singles.tile([128, 1], F32, name="eps2_t", tag="eps2_t")
            nc.vector.memset(eps2_t, 1e-5)

            g1t = load_param(g1_ext, NBLK, "g1t")
            b1t = load_param(b1_ext, NBLK, "b1t")
            g2t = load_param(g2_ext, NBLK, "g2t")
            b2t = load_param(b2_ext, NBLK, "b2t")
            bf1t = load_param(bf1_ext, NJ, "bf1t")
            bf2t = load_param(bf2_ext, NBLK, "bf2t")

            def pad3(t, b):
                # [128, 30, 30] view of block b
                return t[:, b, :].rearrange("p (r c) -> p r c", c=PW)

            def i4(t, b):
                return (pad3(t, b)[:, 1:29, 1:29]
                        .rearrange("p (two r) c -> p two r c", two=2))

            def i4all(t):
                # [128, 3, 28, 28] interior view across blocks 0-2
                return (t.rearrange("p b (r c) -> p b r c", c=PW)
                        [:, 0:3, 1:29, 1:29])

            def i4allx(t):
                # same, but F32-typed view of an f32r tile
                return (t[:, :, :].bitcast(F32)
                        .rearrange("p b (r c) -> p b r c", c=PW)
                        [:, 0:3, 1:29, 1:29])

            def i4x(t, b):
                return (t[:, b, :].bitcast(F32)
                        .rearrange("p (r c) -> p r c", c=PW)[:, 1:29, 1:29]
                        .rearrange("p (two r) c -> p two r c", two=2))

            def ps4(ps):
                return (ps.rearrange("p (two x) -> p two x", two=2)
                        [:, :, 0:N14]
                        .rearrange("p two (r c) -> p two r c", c=28))

            def ps2(ps):
                return (ps.rearrange("p (two x) -> p two x", two=2)
                        [:, :, 0:N14])

            def layernorm(src, dst_slice_fn, eps, gt, bt, post):
                """Channel LN over the 3 partition blocks of `src`
                [128,NBLK,PLEN] (f32r AP) on range [R0, R0+RL)."""
                s1 = psA.tile([128, 1024], F32, tag="conv")
                for (c0, cn) in RCH:
                    for b in range(NBLK):
                        nc.tensor.matmul(
                            out=s1[:, c0:c0 + cn],
                            lhsT=ones_r,
                            rhs=src[:, b, R0 + c0: R0 + c0 + cn],
                            start=(b == 0),
                            stop=(b == NBLK - 1),
                        )
                s2 = psA.tile([128, 1024], F32, tag="conv")
                for b in range(NBLK):
                    sq = stats.tile([128, RL], BF16, tag="sq", bufs=2)
                    nc.scalar.activation(
                        out=sq, in_=src[:, b, R0:R0 + RL].bitcast(F32), func=AF.Square
                    )
                    for (c0, cn) in RCH:
                        nc.tensor.matmul(
                            out=s2[:, c0:c0 + cn],
                            lhsT=ones_b,
                            rhs=sq[:, c0:c0 + cn],
                            start=(b == 0),
                            stop=(b == NBLK - 1),
                        )
                m = stats.tile([128, RL], F32, tag="mstat", bufs=4)
                nc.vector.tensor_scalar_mul(m, s1[:, 0:RL], 1.0 / DIM)
                t2 = stats.tile([128, RL], F32, tag="mstat", bufs=4)
                nc.vector.tensor_scalar_mul(t2, s2[:, 0:RL], 1.0 / DIM)
                msq = stats.tile([128, RL], F32, tag="mstat", bufs=4)
                nc.scalar.activation(out=msq, in_=m, func=AF.Square)
                v = stats.tile([128, RL], F32, tag="mstat", bufs=4)
                nc.vector.tensor_sub(v, t2, msq)
                sd = stats.tile([128, RL], F32, tag="mstat", bufs=4)
                nc.scalar.activation(out=sd, in_=v, func=AF.Sqrt, bias=eps)
                rstd = stats.tile([128, RL], F32, tag="mstat", bufs=4)
                nc.vector.reciprocal_approx_fast(out=rstd, in_=sd)

                if post == "ln1":
                    z0s = []
                    for b in range(NBLK):
                        d = stats.tile([128, RL], F32, tag="dtmp", bufs=2)
                        nc.vector.tensor_sub(d, src[:, b, R0:R0 + RL].bitcast(F32), m)
                        xn = stats.tile([128, RL], BF16, tag="dtmp2", bufs=1)
                        nc.vector.tensor_mul(xn, d, rstd)
                        aff = stats.tile([128, RL], F32, tag="dtmp", bufs=2)
                        nc.vector.tensor_scalar(
                            aff, xn, gt[:, b:b + 1], bt[:, b:b + 1], op.mult, op.add
                        )
                        z0 = stats.tile([128, RL], BF16, tag="z0", bufs=3)
                        nc.vector.tensor_scalar_max(z0, aff, EPS)
                        z0s.append(z0)
                    s0ps = psB.tile([128, 1024], F32, tag="sum")
                    for (c0, cn) in RCH:
                        for b in range(NBLK):
                            nc.tensor.matmul(
                                out=s0ps[:, c0:c0 + cn],
                                lhsT=ones_sc,
                                rhs=z0s[b][:, c0:c0 + cn],
                                start=(b == 0),
                                stop=(b == NBLK - 1),
                            )
                    rs = stats.tile([128, RL], F32, tag="rcpln", bufs=1)
                    nc.vector.reciprocal_approx_fast(out=rs, in_=s0ps[:, 0:RL])
                    for b in range(NBLK):
                        nc.vector.scalar_tensor_tensor(
                            out=dst_slice_fn(b), in0=rs, scalar=1e6 * SC,
                            in1=z0s[b], op0=op.min, op1=op.mult,
                        )
                else:
                    for b in range(NBLK):
                        d = stats.tile([128, RL], F32, tag="dtmp", bufs=2)
                        nc.vector.tensor_sub(d, src[:, b, R0:R0 + RL].bitcast(F32), m)
                        xn = stats.tile([128, RL], BF16, tag="dtmp2", bufs=1)
                        nc.vector.tensor_mul(xn, d, rstd)
                        nc.vector.tensor_scalar(
                            dst_slice_fn(b), xn, gt[:, b:b + 1], bt[:, b:b + 1],
                            op.mult, op.add,
                        )

            # ================= per image (pairs interleaved) =================
            def setup_image(img):
                xpad = pimg.tile([128, NBLK, PLEN], F32R, tag="xpad", bufs=2,
                                 name=f"xpad{img}")
                xnn = pimg.tile([128, NBLK, PLEN], F32R, tag="xnn", bufs=2,
                                name=f"xnn{img}")
                hT = pimg.tile([128, 4, PLEN], FP8, tag="h8", bufs=2,
                               name=f"h8{img}")
                nuT = pimg.tile([128, 4, PLEN], FP8, tag="nu8", bufs=2,
                                name=f"nu8{img}")
                hB = pimg.tile([128, 4, PLEN], BF16, tag="hb", bufs=2,
                               name=f"hb{img}")
                uB = pimg.tile([128, 4, PLEN], BF16, tag="ub", bufs=2,
                               name=f"ub{img}")
                nc.gpsimd.memset(hT, 0.0)
                nc.gpsimd.memset(nuT, 0.0)
                nc.gpsimd.memset(hB, 0.0)
                nc.gpsimd.memset(uB, 0.0)
                nc.gpsimd.memset(hT[:, 3, :], 1.0)
                nc.gpsimd.memset(hB[:, 3, :], 1.0)
                nc.gpsimd.memset(uB[:, 3, :], 5.12e-4)
                for b in range(NBLK):
                    nc.sync.dma_start(
                        out=pad3(xpad, b)[:, 1:29, 1:29],
                        in_=x_ext[img, b * 128:(b + 1) * 128, :, :],
                    )
                    # h init: h = 1/DIM -> h_s = SH/DIM = 1/3
                    nc.gpsimd.memset(pad3(hT, b)[:, 1:29, 1:29], 1.0 / NBLK)
                layernorm(
                    xpad, lambda b: xnn[:, b, R0:R0 + RL], eps1_t, g1t, b1t, "ln1"
                )
                return xpad, xnn, hT, nuT, hB, uB

            def conv_dr(dst_ps, wtile, srcbig, b, aux_delta):
                """3x3 grouped conv on fp8 `srcbig` [128, 4*30, 30] view via 5
                DoubleRow matmuls per half; pair 4 = (tap8, aux at
                +aux_delta with tap-9 weights)."""
                for (r0, c0) in HL:
                    base = b * PW + r0
                    for pair in range(5):
                        if pair < 4:
                            ky0, kx0, delta = PAIR_D[pair]
                        else:
                            ky0, kx0, delta = 2, 2, aux_delta
                        rhs = srcbig[:, base + ky0 - 1: base + ky0 + 13,
                                     kx0: kx0 + 28].unsqueeze(1)
                        rhs.ap[1] = [delta, 2]
                        nc.tensor.matmul(
                            out=dst_ps[:, c0:c0 + N14],
                            lhsT=wtile[:, 2 * pair: 2 * pair + 2, :],
                            rhs=rhs,
                            start=(pair == 0),
                            stop=(pair == 4),
                            perf_mode=PM.DoubleRow,
                        )

            def iter8_a(ts):
                xpad, xnn, hT, nuT, hB, uB = ts
                hbig = hT.rearrange("p b (r c) -> p (b r) c", c=PW)
                # recon = conv(h, wt) + eps ; rcp = exp(-ln(recon_ps)) on Act
                lnt = stats.tile([128, NBLK, 784], F16, tag="lnt", bufs=1)
                for b in range(NBLK):
                    ps = psA.tile([128, 1024], F32, tag="conv")
                    conv_dr(ps, wb8[b], hbig, b, (3 - b) * PLEN)
                    nc.scalar.activation(
                        out=lnt[:, b, :].rearrange("p (two x) -> p two x", two=2),
                        in_=ps2(ps), func=AF.Ln,
                    )
                rcp = stats.tile([128, NBLK, 784], BF16, tag="rcp", bufs=2)
                nc.scalar.activation(
                    out=rcp.rearrange("p b x -> p (b x)"),
                    in_=lnt.rearrange("p b x -> p (b x)"),
                    func=AF.Exp, scale=-1.0,
                )
                # nu_s = min(rcp, C1) * xnn_s   (per-block stt, fp8 out)
                for b in range(NBLK):
                    rcp4b = rcp[:, b, :].rearrange(
                        "p (two r c) -> p two r c", two=2, c=28)
                    nc.vector.scalar_tensor_tensor(
                        out=i4(nuT, b), in0=rcp4b, scalar=C1,
                        in1=i4x(xnn, b), op0=op.min, op1=op.mult,
                    )
            def iter8_b(ts, last):
                xpad, xnn, hT, nuT, hB, uB = ts
                nbig = nuT.rearrange("p b (r c) -> p (b r) c", c=PW)
                # u_b = fwd_ps . h_s = 65536*u  -> bf16 (scale cancels later)
                for b in range(NBLK):
                    ps = psA.tile([128, 1024], F32, tag="conv")
                    conv_dr(ps, wf8[b], nbig, b, PLEN)
                    nc.vector.tensor_mul(i4(uB, b), ps4(ps), i4(hT, b))
                # colsum over 4 blocks (block 3 = 1e-6 eps) ; sinv = 1/cs
                ss = psB.tile([128, 1024], F32, tag="sum")
                for (r0, c0) in HL:
                    for b in range(4):
                        nc.tensor.matmul(
                            out=ss[:, c0:c0 + N14],
                            lhsT=ones_cs,
                            rhs=pad3(uB, b)[:, r0:r0 + 14, 1:29],
                            start=(b == 0),
                            stop=(b == 3),
                        )
                lnt2 = stats.tile([128, 784], F16, tag="lnt2", bufs=1)
                nc.scalar.activation(
                    out=lnt2.rearrange("p (two x) -> p two x", two=2),
                    in_=ps2(ss), func=AF.Ln,
                )
                sinv = stats.tile([128, 784], BF16, tag="sinv", bufs=2)
                nc.scalar.activation(out=sinv, in_=lnt2, func=AF.Exp, scale=-1.0)
                sinv4 = sinv.rearrange("p (two r c) -> p two r c", two=2, c=28)
                hdst = hB if last else hT
                for b in range(NBLK):
                    nc.gpsimd.tensor_mul(i4(hdst, b), sinv4, i4(uB, b))

            def iter16_a(ts):
                xpad, xnn, hT, nuT, hB, uB = ts
                # recon (bf16, 9 taps + eps tap reading hB block 3)
                for b in range(NBLK):
                    h3 = pad3(hB, b)
                    ps = psA.tile([128, 1024], F32, tag="conv")
                    for (r0, c0) in HL:
                        for t in range(9):
                            ky, kx = t // 3, t % 3
                            nc.tensor.matmul(
                                out=ps[:, c0:c0 + N14],
                                lhsT=wbwd[b][:, t, :],
                                rhs=h3[:, r0 + ky - 1: r0 + ky + 13,
                                       kx: kx + 28],
                                start=(t == 0),
                                stop=False,
                            )
                        nc.tensor.matmul(
                            out=ps[:, c0:c0 + N14],
                            lhsT=wbwd[b][:, 9, :],
                            rhs=pad3(hB, 3)[:, r0:r0 + 14, 1:29],
                            start=False,
                            stop=True,
                        )
                    rcpb = stats.tile([128, RL], F32, tag="mstat", bufs=4)
                    nc.vector.reciprocal_approx_fast(
                        out=rcpb[:, 0:784].rearrange("p (two x) -> p two x", two=2),
                        in_=ps2(ps),
                    )
                    # nu_t = rcp * xnn_s -> bf16 (1e6 cap automatic via eps tap)
                    nc.gpsimd.tensor_mul(
                        i4(uB, b),
                        rcpb[:, 0:784].rearrange(
                            "p (two r c) -> p two r c", two=2, c=28),
                        i4x(xnn, b),
                    )
            def iter16_b(ts):
                xpad, xnn, hT, nuT, hB, uB = ts
                for b in range(NBLK):
                    nu3 = pad3(uB, b)
                    ps = psA.tile([128, 1024], F32, tag="conv")
                    for (r0, c0) in HL:
                        for t in range(9):
                            ky, kx = t // 3, t % 3
                            nc.tensor.matmul(
                                out=ps[:, c0:c0 + N14],
                                lhsT=wfwd[b][:, t, :],
                                rhs=nu3[:, r0 + ky - 1: r0 + ky + 13,
                                        kx: kx + 28],
                                start=(t == 0),
                                stop=(t == 8),
                            )
                    nc.vector.tensor_mul(i4(uB, b), ps4(ps), i4(hB, b))
                # h = u / colsum  (x SH)
                ss = psB.tile([128, 1024], F32, tag="sum")
                for (r0, c0) in HL:
                    for b in range(4):
                        nc.tensor.matmul(
                            out=ss[:, c0:c0 + N14],
                            lhsT=ones_cs,
                            rhs=pad3(uB, b)[:, r0:r0 + 14, 1:29],
                            start=(b == 0),
                            stop=(b == 3),
                        )
                sinvt = stats.tile([128, RL], F32, tag="mstat", bufs=4)
                nc.vector.reciprocal_approx_fast(
                    out=sinvt[:, 0:784].rearrange("p (two x) -> p two x", two=2),
                    in_=ps2(ss),
                )
                sinv4 = sinvt[:, 0:784].rearrange(
                    "p (two r c) -> p two r c", two=2, c=28)
                for b in range(NBLK):
                    nc.gpsimd.tensor_mul(i4(hB, b), sinv4, i4(uB, b))

            def tail_ln(img, ts):
                xpad, xnn, hT, nuT, hB, uB = ts
                # residual: x2 = x + h  (h stored x SH) -> into xnn slot
                for b in range(NBLK):
                    nc.vector.scalar_tensor_tensor(
                        out=xnn[:, b, R0:R0 + RL],
                        in0=hB[:, b, R0:R0 + RL], scalar=1.0 / SH,
                        in1=xpad[:, b, R0:R0 + RL].bitcast(F32),
                        op0=op.mult, op1=op.add,
                    )
                # LN2 -> xn2 (into xpad slot)
                layernorm(
                    xnn,
                    lambda b: xpad[:, b, R0:R0 + RL],
                    eps2_t, g2t, b2t, "ln2",
                )

            def tail_mlp(img, ts):
                xpad, xnn, hT, nuT, hB, uB = ts
                hid = pimg.tile([128, NJ, RL], BF16, tag="hid", bufs=1,
                                name=f"hid{img}")
                for j in range(NJ):
                    hp = psA.tile([128, 1024], F32, tag="conv")
                    for (c0, cn) in RCH:
                        for kb in range(NBLK):
                            nc.tensor.matmul(
                                out=hp[:, c0:c0 + cn],
                                lhsT=w1t[kb][:, j, :],
                                rhs=xpad[:, kb, R0 + c0: R0 + c0 + cn],
                                start=(kb == 0),
                                stop=(kb == NBLK - 1),
                            )
                    nc.scalar.activation(
                        out=hid[:, j, :], in_=hp[:, 0:RL], func=AF.Gelu,
                        bias=bf1t[:, j:j + 1], scale=1.0,
                    )
                for cb in range(NBLK):
                    ops_ = psB.tile([128, 1024], F32, tag="sum")
                    for (c0, cn) in RCH:
                        for j in range(NJ):
                            nc.tensor.matmul(
                                out=ops_[:, c0:c0 + cn],
                                lhsT=w2t[j][:, cb, :],
                                rhs=hid[:, j, c0:c0 + cn],
                                start=(j == 0),
                                stop=(j == NJ - 1),
                            )
                    nc.vector.scalar_tensor_tensor(
                        out=xpad[:, cb, R0:R0 + RL],
                        in0=ops_[:, 0:RL],
                        scalar=bf2t[:, cb:cb + 1],
                        in1=xnn[:, cb, R0:R0 + RL].bitcast(F32),
                        op0=op.add, op1=op.add,
                    )
                for b in range(NBLK):
                    src = (xpad[:, b, :].bitcast(F32)
                           .rearrange("p (r c) -> p r c", c=PW)[:, 1:29, 1:29])
                    nc.sync.dma_start(
                        out=out_ext[img, b * 128:(b + 1) * 128, :, :],
                        in_=src,
                    )

            for p0 in range(0, NB, 2):
                imgs = list(range(p0, min(p0 + 2, NB)))
                tsets = {img: setup_image(img) for img in imgs}
                for it in range(N8):
                    for img in imgs:
                        iter8_a(tsets[img])
                    for img in imgs:
                        iter8_b(tsets[img], last=(it == N8 - 1))
                for it in range(NB16):
                    for img in imgs:
                        iter16_a(tsets[img])
                    for img in imgs:
                        iter16_b(tsets[img])
                for img in imgs:
                    tail_ln(img, tsets[img])
                for img in imgs:
                    tail_mlp(img, tsets[img])

    try:
        nc.compile()
    finally:
        bacc.get_activation_tables = _orig_tables
    return nc


def _prep_weights(Wc, g1, b1, g2, b2, w_fc1, b_fc1, w_fc2, b_fc2):
    import ml_dtypes

    wp = np.abs(np.asarray(Wc, np.float32))
    wp = wp / np.maximum(wp.sum(axis=(1, 2, 3), keepdims=True), EPS)
    wp4 = wp.reshape(NBLK, 2, 64, 64, 3, 3)  # [b, gi, co, ci, ky, kx]
    afwd = np.zeros((NBLK, 128, 9, 128), np.float32)
    abwd = np.zeros((NBLK, 128, 10, 128), np.float32)
    for b in range(NBLK):
        for gi in range(2):
            blk = wp4[b, gi]
            afwd[b, gi * 64:(gi + 1) * 64, :, gi * 64:(gi + 1) * 64] = (
                blk.transpose(1, 2, 3, 0).reshape(64, 9, 64)
            )
            abwd[b, gi * 64:(gi + 1) * 64, 0:9, gi * 64:(gi + 1) * 64] = (
                blk[:, :, ::-1, ::-1].transpose(0, 2, 3, 1).reshape(64, 9, 64)
            )
    abwd[:, 0, 9, :] = EPSWT
    wf8 = np.zeros((NBLK, 128, 10, 128), np.float32)
    wf8[:, :, 0:9, :] = afwd * SWF
    wb8 = abwd * SWB
    wb8[:, :, 9, :] = 0.0
    wb8[:, 0, 9, :] = EPSW8
    wf8 = wf8.astype(ml_dtypes.float8_e4m3)
    wb8 = wb8.astype(ml_dtypes.float8_e4m3)
    assert np.isfinite(wf8.astype(np.float32)).all()
    assert np.isfinite(wb8.astype(np.float32)).all()
    w1 = np.asarray(w_fc1, np.float32).reshape(NBLK, 128, NJ, 128)
    w2 = np.asarray(w_fc2, np.float32).reshape(NJ, 128, NBLK, 128).astype(
        ml_dtypes.bfloat16
    )
    return {
        "afwd": (afwd / 32.0).astype(ml_dtypes.bfloat16),
        "abwd": abwd.astype(ml_dtypes.bfloat16),
        "wf8": wf8,
        "wb8": wb8,
        "w1": _round_tf32(w1),
        "w2": w2,
        "g1": np.asarray(g1, np.float32).reshape(NBLK, 128),
        "b1": np.asarray(b1, np.float32).reshape(NBLK, 128),
        "g2": np.asarray(g2, np.float32).reshape(NBLK, 128),
        "b2": np.asarray(b2, np.float32).reshape(NBLK, 128),
        "bf1": np.asarray(b_fc1, np.float32).reshape(NJ, 128),
        "bf2": np.asarray(b_fc2, np.float32).reshape(NBLK, 128),
    }


_last_result = None


def kernel(x, g1, b1, Wc, g2, b2, w_fc1, b_fc1, w_fc2, b_fc2):
    global _last_result
    if os.environ.get("JAX_PLATFORMS", "").strip().lower() == "cpu":
        del os.environ["JAX_PLATFORMS"]
    from concourse.bass_utils import run_bass_kernel_spmd

    if "nc" not in _cache:
        _cache["nc"] = _build()
    nc = _cache["nc"]

    shared = _prep_weights(Wc, g1, b1, g2, b2, w_fc1, b_fc1, w_fc2, b_fc2)
    x = np.asarray(x, np.float32)
    assert x.shape == (NB * NCORES, DIM, H, W), x.shape
    in_maps = []
    for c in range(NCORES):
        m = dict(shared)
        m["x"] = np.ascontiguousarray(x[c * NB:(c + 1) * NB])
        in_maps.append(m)

    r = run_bass_kernel_spmd(
        nc, in_maps, list(range(NCORES)),
        trace=bool(os.environ.get("K_TRACE")),
    )
    _last_result = r
    out = np.concatenate(
        [r.results[c]["out"] for c in range(NCORES)], axis=0
    ).astype(np.float32)
    return out


# revision 17
# speedup vs baseline: 1.6544x; 1.1277x over previous
"""Trainium2 Bass kernel for nn_Block_34711925686730 (dense_transformer).

Pipeline per image (data-parallel over batch, 4 images / NeuronCore):
  LN(channels) -> iterative KL-NNMF grouped conv (25 iters) -> residual
  -> LN(channels) -> MLP (gelu) -> residual.

NNMF runs in two phases:
  * N8 fp8 iterations: grouped 3x3 convs as fp8e4m3 DoubleRow matmuls
    (2 taps contracted per instruction at 0.5 PE-cycles/row).  Operands
    are power-of-2 pre-scaled (dict x2048/x64, h x128, nu x8) so values
    sit in e4m3's normal range; an extra "eps tap" (0.25 into row 0,
    reading a constant-1.0 h-block) adds the reference's 1e-6 recon
    floor for free and makes the reciprocals NaN-safe.  recon recip
    runs on the Act engine as exp(-ln(x)); nu/u/h updates are
    scalar_tensor_tensor ops split across DVE and GpSimd.
  * NB16 bf16 tail iterations (accurate dict) contract the fp8-phase
    perturbation back toward the f32 fixed point; h is stored x128 in
    both phases so the hand-off is a plain buffer switch.

Layout: channels on partitions (3 blocks of 128 = 2 conv groups of 64),
spatial flattened into a zero-padded 30x30 = 900-col free dim per image.
LN stat sums and fc1 run in fp32r (tf32); fc2 and the tail convs in
bf16.  Channel reductions use ones-matmuls that also broadcast the sum
to all partitions.
"""

import os
import numpy as np

DIM = 384
HEADS = 6
N8 = int(os.environ.get("K_N8", "20"))      # fp8 NNMF iterations
NB16 = int(os.environ.get("K_NB16", "5"))   # bf16 tail iterations
NB = int(os.environ.get("K_NB", "4"))       # images per core
MLP_HID = 4 * DIM
EPS = 1e-6
H = W = 28
NCORES = 8
NBLK = 3          # channel blocks of 128
PW = 30           # padded width
PLEN = 900        # padded spatial length (30*30)
S0 = 31           # first conv-output flat index (y=1,x=1)
R0 = 30           # stats/MLP range start (even, = (1,0))
RL = 840          # stats/MLP columns [30, 870)
NJ = MLP_HID // 128  # 12
N14 = 14 * 28

# fp8 scales (all powers of two; folded exactly through the pipeline)
SWB = 2048.0      # recon dictionary scale
SWF = 64.0        # fwd dictionary scale
SNU = 8.0         # nu storage scale
SH = 128.0        # h storage scale (fp8 phase AND bf16 tail)
SC = SNU * SWB * SH          # 2^21: xnn_s = SC * xnn
C1 = 480.0 / (SWB * SH)      # fp8-phase recip clamp (ratio cap 480)
C3 = 1e6 / SH                # tail recip clamp (reference 1e6 cap)
KU8 = 1.0 / (SWF * SNU)      # fp8-phase u scalar  (2^-9)
KUT = SH / SC                # tail u scalar       (2^-14)
EPSW8 = 0.25                 # fp8 eps tap value -> recon floor ~0.95e-6
EPSWT = SH * 1e-6            # tail eps tap value -> recon floor 1e-6

_cache = {}


def _round_tf32(a):
    bits = np.ascontiguousarray(a, dtype=np.float32).view(np.uint32)
    r = bits + np.uint32(0x0FFF) + ((bits >> np.uint32(13)) & np.uint32(1))
    return (r & np.uint32(0xFFFFE000)).view(np.float32).copy()


def _build():
    import concourse.bacc as bacc
    import concourse.mybir as mybir
    import concourse.tile as tile
    from concourse.hw_specs import get_activation_tables

    # The act-table chooser is first-match over act_info.json order, which
    # ping-pongs 1283ns table loads between 'natural_log' and
    # 'exp_and_others' for our exp(-ln(x)) reciprocals.  Neuter those two
    # single-function tables (ids keep their positions) so both Ln and Exp
    # resolve to 'natural_log_exp_and_others'.
    _orig_tables = bacc.get_activation_tables

    def _patched_tables(arch):
        tabs = dict(get_activation_tables(arch))
        if "natural_log_exp_and_others" in tabs:
            for name in ("exp_and_others", "natural_log"):
                if name in tabs:
                    tabs[name] = set()
        return tabs

    bacc.get_activation_tables = _patched_tables

    F32 = mybir.dt.float32
    F32R = mybir.dt.float32r
    BF16 = mybir.dt.bfloat16
    FP8 = mybir.dt.float8e4
    F16 = mybir.dt.float16
    AF = mybir.ActivationFunctionType
    op = mybir.AluOpType
    PM = mybir.MatmulPerfMode

    nc = bacc.Bacc("TRN2", target_bir_lowering=False, debug=False)

    x_ext = nc.declare_dram_parameter("x", [NB, DIM, H, W], F32R, isOutput=False)
    afwd_ext = nc.declare_dram_parameter("afwd", [NBLK, 128, 9, 128], BF16, isOutput=False)
    abwd_ext = nc.declare_dram_parameter("abwd", [NBLK, 128, 10, 128], BF16, isOutput=False)
    wf8_ext = nc.declare_dram_parameter("wf8", [NBLK, 128, 10, 128], FP8, isOutput=False)
    wb8_ext = nc.declare_dram_parameter("wb8", [NBLK, 128, 10, 128], FP8, isOutput=False)
    w1q_ext = nc.declare_dram_parameter("w1q", [2, 128, NJ, 2, 128], FP8, isOutput=False)
    w2q_ext = nc.declare_dram_parameter("w2q", [128, 6, NBLK, 2, 128], FP8, isOutput=False)
    bf2b_ext = nc.declare_dram_parameter("bf2b", [1, NBLK, 128], BF16, isOutput=False)
    g1_ext = nc.declare_dram_parameter("g1", [NBLK, 128], F32, isOutput=False)
    b1_ext = nc.declare_dram_parameter("b1", [NBLK, 128], F32, isOutput=False)
    g2_ext = nc.declare_dram_parameter("g2", [NBLK, 128], F32, isOutput=False)
    b2_ext = nc.declare_dram_parameter("b2", [NBLK, 128], F32, isOutput=False)
    bf1_ext = nc.declare_dram_parameter("bf1", [NJ, 128], F32, isOutput=False)
    bf2_ext = nc.declare_dram_parameter("bf2", [NBLK, 128], F32, isOutput=False)
    out_ext = nc.declare_dram_parameter("out", [NB, DIM, H, W], F32, isOutput=True)

    # PSUM accumulation groups must not cross a 512-col (2KB) bank boundary
    RCH = [(0, 512), (512, RL - 512)]   # stats/MLP chunks (within [R0, R0+RL))
    HL = [(1, 0), (15, 512)]            # (first interior row, psum col)
    # DR tap pairs: (0,1),(2,3),(4,5),(6,7),(8,aux).  The aux read of the
    # last pair is stride `d8` away; its weights live in tap slot 9.
    PAIR_D = []
    for pair in range(4):
        t0 = 2 * pair
        ky0, kx0 = t0 // 3, t0 % 3
        ky1, kx1 = (t0 + 1) // 3, (t0 + 1) % 3
        PAIR_D.append((ky0, kx0, (ky1 - ky0) * PW + (kx1 - kx0)))

    with tile.TileContext(nc) as tc:
        with (
            tc.tile_pool(name="singles", bufs=1) as singles,
            tc.tile_pool(name="img", bufs=1) as pimg,
            tc.tile_pool(name="stats", bufs=3) as stats,
            tc.tile_pool(name="psA", bufs=3, space="PSUM") as psA,
            tc.tile_pool(name="psB", bufs=1, space="PSUM") as psB,
        ):
            # ---- weights / params resident in SBUF ----
            onesf = singles.tile([128, 128], F32)
            nc.vector.memset(onesf, 1.0)
            ones_r = singles.tile([128, 128], F32R)
            nc.vector.tensor_copy(ones_r, onesf)
            ones_b = singles.tile([128, 128], BF16)
            nc.vector.memset(ones_b, 1.0)
            ones_sc = singles.tile([128, 128], BF16, name="ones_sc", tag="ones_sc")
            nc.vector.memset(ones_sc, 1.0 / SC)
            ones_cs = singles.tile([128, 128], BF16, name="ones_cs", tag="ones_cs")
            nc.vector.memset(ones_cs, 1.0 / SH)
            wfwd = []
            wbwd = []
            wf8 = []
            wb8 = []
            for b in range(NBLK):
                wf = singles.tile([128, 9, 128], BF16, name=f"wfwd{b}", tag=f"wfwd{b}")
                nc.sync.dma_start(out=wf, in_=afwd_ext[b])
                wfwd.append(wf)
                wb = singles.tile([128, 10, 128], BF16, name=f"wbwd{b}", tag=f"wbwd{b}")
                nc.sync.dma_start(out=wb, in_=abwd_ext[b])
                wbwd.append(wb)
                t8 = singles.tile([128, 10, 128], FP8, name=f"wf8{b}", tag=f"wf8{b}")
                nc.sync.dma_start(out=t8, in_=wf8_ext[b])
                wf8.append(t8)
                t8 = singles.tile([128, 10, 128], FP8, name=f"wb8{b}", tag=f"wb8{b}")
                nc.sync.dma_start(out=t8, in_=wb8_ext[b])
                wb8.append(t8)
            w1qt = []
            for p in range(2):
                t = singles.tile([128, NJ, 2, 128], FP8, name=f"w1q{p}", tag=f"w1q{p}")
                nc.sync.dma_start(out=t, in_=w1q_ext[p])
                w1qt.append(t)
            w2qt = singles.tile([128, 6, NBLK, 2, 128], FP8, name="w2qt", tag="w2qt")
            nc.sync.dma_start(out=w2qt, in_=w2q_ext[:, :, :, :, :])
            bf2row = singles.tile([1, NBLK, 128], BF16, name="bf2row", tag="bf2row")
            nc.sync.dma_start(out=bf2row, in_=bf2b_ext[0])
            ones_row = singles.tile([1, RL], BF16, name="ones_row", tag="ones_row")
            nc.vector.memset(ones_row, 1.0)

            def load_param(ext, n, name):
                t = singles.tile([128, n], F32, name=name, tag=name)
                nc.sync.dma_start(out=t, in_=ext[:, :].rearrange("b p -> p b"))
                return t

            eps1_t = singles.tile([128, 1], F32, name="eps1_t", tag="eps1_t")
            nc.vector.memset(eps1_t, EPS)
            eps2_t = singles.tile([128, 1], F32, name="eps2_t", tag="eps2_t")
            nc.vector.memset(eps2_t, 1e-5)

            g1t = load_param(g1_ext, NBLK, "g1t")
            b1t = load_param(b1_ext, NBLK, "b1t")
            g2t = load_param(g2_ext, NBLK, "g2t")
            b2t = load_param(b2_ext, NBLK, "b2t")
            bf1t = load_param(bf1_ext, NJ, "bf1t")
            bf2t = load_param(bf2_ext, NBLK, "bf2t")

            def pad3(t, b):
                # [128, 30, 30] view of block b
                return t[:, b, :].rearrange("p (r c) -> p r c", c=PW)

            def i4(t, b):
                return (pad3(t, b)[:, 1:29, 1:29]
                        .rearrange("p (two r) c -> p two r c", two=2))

            def i4all(t):
                # [128, 3, 28, 28] interior view across blocks 0-2
                return (t.rearrange("p b (r c) -> p b r c", c=PW)
                        [:, 0:3, 1:29, 1:29])

            def i4allx(t):
                # same, but F32-typed view of an f32r tile
                return (t[:, :, :].bitcast(F32)
                        .rearrange("p b (r c) -> p b r c", c=PW)
                        [:, 0:3, 1:29, 1:29])

            def i4x(t, b):
                return (t[:, b, :].bitcast(F32)
                        .rearrange("p (r c) -> p r c", c=PW)[:, 1:29, 1:29]
                        .rearrange("p (two r) c -> p two r c", two=2))

            def ps4(ps):
                return (ps.rearrange("p (two x) -> p two x", two=2)
                        [:, :, 0:N14]
                        .rearrange("p two (r c) -> p two r c", c=28))

            def ps2(ps):
                return (ps.rearrange("p (two x) -> p two x", two=2)
                        [:, :, 0:N14])

            def layernorm(src, dst_slice_fn, eps, gt, bt, post):
                """Channel LN over the 3 partition blocks of `src`
                [128,NBLK,PLEN] (f32r AP) on range [R0, R0+RL)."""
                s1 = psA.tile([128, 1024], F32, tag="conv")
                for (c0, cn) in RCH:
                    for b in range(NBLK):
                        nc.tensor.matmul(
                            out=s1[:, c0:c0 + cn],
                            lhsT=ones_r,
                            rhs=src[:, b, R0 + c0: R0 + c0 + cn],
                            start=(b == 0),
                            stop=(b == NBLK - 1),
                        )
                s2 = psA.tile([128, 1024], F32, tag="conv")
                for b in range(NBLK):
                    sq = stats.tile([128, RL], BF16, tag="sq", bufs=2)
                    nc.scalar.activation(
                        out=sq, in_=src[:, b, R0:R0 + RL].bitcast(F32), func=AF.Square
                    )
                    for (c0, cn) in RCH:
                        nc.tensor.matmul(
                            out=s2[:, c0:c0 + cn],
                            lhsT=ones_b,
                            rhs=sq[:, c0:c0 + cn],
                            start=(b == 0),
                            stop=(b == NBLK - 1),
                        )
                m = stats.tile([128, RL], F32, tag="mstat", bufs=4)
                nc.vector.tensor_scalar_mul(m, s1[:, 0:RL], 1.0 / DIM)
                t2 = stats.tile([128, RL], F32, tag="mstat", bufs=4)
                nc.vector.tensor_scalar_mul(t2, s2[:, 0:RL], 1.0 / DIM)
                msq = stats.tile([128, RL], F32, tag="mstat", bufs=4)
                nc.scalar.activation(out=msq, in_=m, func=AF.Square)
                v = stats.tile([128, RL], F32, tag="mstat", bufs=4)
                nc.vector.tensor_sub(v, t2, msq)
                sd = stats.tile([128, RL], F32, tag="mstat", bufs=4)
                nc.scalar.activation(out=sd, in_=v, func=AF.Sqrt, bias=eps)
                rstd = stats.tile([128, RL], F32, tag="mstat", bufs=4)
                nc.vector.reciprocal_approx_fast(out=rstd, in_=sd)

                if post == "ln1":
                    z0s = []
                    for b in range(NBLK):
                        d = stats.tile([128, RL], F32, tag="dtmp", bufs=2)
                        nc.vector.tensor_sub(d, src[:, b, R0:R0 + RL].bitcast(F32), m)
                        xn = stats.tile([128, RL], BF16, tag="dtmp2", bufs=1)
                        nc.vector.tensor_mul(xn, d, rstd)
                        aff = stats.tile([128, RL], F32, tag="dtmp", bufs=2)
                        nc.vector.tensor_scalar(
                            aff, xn, gt[:, b:b + 1], bt[:, b:b + 1], op.mult, op.add
                        )
                        z0 = stats.tile([128, RL], BF16, tag="z0", bufs=3)
                        nc.vector.tensor_scalar_max(z0, aff, EPS)
                        z0s.append(z0)
                    s0ps = psB.tile([128, 1024], F32, tag="sum")
                    for (c0, cn) in RCH:
                        for b in range(NBLK):
                            nc.tensor.matmul(
                                out=s0ps[:, c0:c0 + cn],
                                lhsT=ones_sc,
                                rhs=z0s[b][:, c0:c0 + cn],
                                start=(b == 0),
                                stop=(b == NBLK - 1),
                            )
                    rs = stats.tile([128, RL], F32, tag="rcpln", bufs=1)
                    nc.vector.reciprocal_approx_fast(out=rs, in_=s0ps[:, 0:RL])
                    for b in range(NBLK):
                        nc.vector.scalar_tensor_tensor(
                            out=dst_slice_fn(b), in0=rs, scalar=1e6 * SC,
                            in1=z0s[b], op0=op.min, op1=op.mult,
                        )
                else:
                    for b in range(NBLK):
                        d = stats.tile([128, RL], F32, tag="dtmp", bufs=2)
                        nc.vector.tensor_sub(d, src[:, b, R0:R0 + RL].bitcast(F32), m)
                        xn = stats.tile([128, RL], BF16, tag="dtmp2", bufs=1)
                        nc.vector.tensor_mul(xn, d, rstd)
                        nc.vector.tensor_scalar(
                            dst_slice_fn(b), xn, gt[:, b:b + 1], bt[:, b:b + 1],
                            op.mult, op.add,
                        )

            # ================= per image (pairs interleaved) =================
            def setup_image(img):
                xpad = pimg.tile([128, NBLK, PLEN], F32R, tag="xpad", bufs=2,
                                 name=f"xpad{img}")
                xnn = pimg.tile([128, NBLK, PLEN], F32R, tag="xnn", bufs=2,
                                name=f"xnn{img}")
                hT = pimg.tile([128, 4, PLEN], FP8, tag="h8", bufs=2,
                               name=f"h8{img}")
                nuT = pimg.tile([128, 4, PLEN], FP8, tag="nu8", bufs=2,
                                name=f"nu8{img}")
                hB = pimg.tile([128, 4, PLEN], BF16, tag="hb", bufs=2,
                               name=f"hb{img}")
                uB = pimg.tile([128, 4, PLEN], BF16, tag="ub", bufs=2,
                               name=f"ub{img}")
                yq = pimg.tile([128, 4, RL], FP8, tag="yq", bufs=2,
                               name=f"yq{img}")
                nc.gpsimd.memset(yq, 0.0)
                nc.gpsimd.memset(hT, 0.0)
                nc.gpsimd.memset(nuT, 0.0)
                nc.gpsimd.memset(hB, 0.0)
                nc.gpsimd.memset(uB, 0.0)
                nc.gpsimd.memset(hT[:, 3, :], 1.0)
                nc.gpsimd.memset(hB[:, 3, :], 1.0)
                nc.gpsimd.memset(uB[:, 3, :], 5.12e-4)
                for b in range(NBLK):
                    nc.sync.dma_start(
                        out=pad3(xpad, b)[:, 1:29, 1:29],
                        in_=x_ext[img, b * 128:(b + 1) * 128, :, :],
                    )
                    # h init: h = 1/DIM -> h_s = SH/DIM = 1/3
                    nc.gpsimd.memset(pad3(hT, b)[:, 1:29, 1:29], 1.0 / NBLK)
                layernorm(
                    xpad, lambda b: xnn[:, b, R0:R0 + RL], eps1_t, g1t, b1t, "ln1"
                )
                return xpad, xnn, hT, nuT, hB, uB, yq

            def conv_dr(dst_ps, wtile, srcbig, b, aux_delta):
                """3x3 grouped conv on fp8 `srcbig` [128, 4*30, 30] view via 5
                DoubleRow matmuls per half; pair 4 = (tap8, aux at
                +aux_delta with tap-9 weights)."""
                for (r0, c0) in HL:
                    base = b * PW + r0
                    for pair in range(5):
                        if pair < 4:
                            ky0, kx0, delta = PAIR_D[pair]
                        else:
                            ky0, kx0, delta = 2, 2, aux_delta
                        rhs = srcbig[:, base + ky0 - 1: base + ky0 + 13,
                                     kx0: kx0 + 28].unsqueeze(1)
                        rhs.ap[1] = [delta, 2]
                        nc.tensor.matmul(
                            out=dst_ps[:, c0:c0 + N14],
                            lhsT=wtile[:, 2 * pair: 2 * pair + 2, :],
                            rhs=rhs,
                            start=(pair == 0),
                            stop=(pair == 4),
                            perf_mode=PM.DoubleRow,
                        )

            def iter8_a(ts):
                xpad, xnn, hT, nuT, hB, uB, yq = ts
                hbig = hT.rearrange("p b (r c) -> p (b r) c", c=PW)
                # recon = conv(h, wt) + eps ; rcp = exp(-ln(recon_ps)) on Act
                lnt = stats.tile([128, NBLK, 784], F16, tag="lnt", bufs=1)
                for b in range(NBLK):
                    ps = psA.tile([128, 1024], F32, tag="conv")
                    conv_dr(ps, wb8[b], hbig, b, (3 - b) * PLEN)
                    nc.scalar.activation(
                        out=lnt[:, b, :].rearrange("p (two x) -> p two x", two=2),
                        in_=ps2(ps), func=AF.Ln,
                    )
                rcp = stats.tile([128, NBLK, 784], BF16, tag="rcp", bufs=2)
                nc.scalar.activation(
                    out=rcp.rearrange("p b x -> p (b x)"),
                    in_=lnt.rearrange("p b x -> p (b x)"),
                    func=AF.Exp, scale=-1.0,
                )
                # nu_s = min(rcp, C1) * xnn_s   (per-block stt, fp8 out)
                for b in range(NBLK):
                    rcp4b = rcp[:, b, :].rearrange(
                        "p (two r c) -> p two r c", two=2, c=28)
                    nc.vector.scalar_tensor_tensor(
                        out=i4(nuT, b), in0=rcp4b, scalar=C1,
                        in1=i4x(xnn, b), op0=op.min, op1=op.mult,
                    )
            def iter8_b(ts, last):
                xpad, xnn, hT, nuT, hB, uB, yq = ts
                nbig = nuT.rearrange("p b (r c) -> p (b r) c", c=PW)
                # u_b = fwd_ps . h_s = 65536*u  -> bf16 (scale cancels later)
                for b in range(NBLK):
                    ps = psA.tile([128, 1024], F32, tag="conv")
                    conv_dr(ps, wf8[b], nbig, b, PLEN)
                    nc.vector.tensor_mul(i4(uB, b), ps4(ps), i4(hT, b))
                # colsum over 4 blocks (block 3 = 1e-6 eps) ; sinv = 1/cs
                ss = psB.tile([128, 1024], F32, tag="sum")
                for (r0, c0) in HL:
                    for b in range(4):
                        nc.tensor.matmul(
                            out=ss[:, c0:c0 + N14],
                            lhsT=ones_cs,
                            rhs=pad3(uB, b)[:, r0:r0 + 14, 1:29],
                            start=(b == 0),
                            stop=(b == 3),
                        )
                lnt2 = stats.tile([128, 784], F16, tag="lnt2", bufs=1)
                nc.scalar.activation(
                    out=lnt2.rearrange("p (two x) -> p two x", two=2),
                    in_=ps2(ss), func=AF.Ln,
                )
                sinv = stats.tile([128, 784], BF16, tag="sinv", bufs=2)
                nc.scalar.activation(out=sinv, in_=lnt2, func=AF.Exp, scale=-1.0)
                sinv4 = sinv.rearrange("p (two r c) -> p two r c", two=2, c=28)
                hdst = hB if last else hT
                for b in range(NBLK):
                    eng = nc.vector if b == 0 else nc.gpsimd
                    eng.tensor_mul(i4(hdst, b), sinv4, i4(uB, b))

            def iter16_a(ts):
                xpad, xnn, hT, nuT, hB, uB, yq = ts
                # recon (bf16, 9 taps + eps tap reading hB block 3)
                for b in range(NBLK):
                    h3 = pad3(hB, b)
                    ps = psA.tile([128, 1024], F32, tag="conv")
                    for (r0, c0) in HL:
                        for t in range(9):
                            ky, kx = t // 3, t % 3
                            nc.tensor.matmul(
                                out=ps[:, c0:c0 + N14],
                                lhsT=wbwd[b][:, t, :],
                                rhs=h3[:, r0 + ky - 1: r0 + ky + 13,
                                       kx: kx + 28],
                                start=(t == 0),
                                stop=False,
                            )
                        nc.tensor.matmul(
                            out=ps[:, c0:c0 + N14],
                            lhsT=wbwd[b][:, 9, :],
                            rhs=pad3(hB, 3)[:, r0:r0 + 14, 1:29],
                            start=False,
                            stop=True,
                        )
                    rcpb = stats.tile([128, RL], F32, tag="mstat", bufs=4)
                    nc.vector.reciprocal_approx_fast(
                        out=rcpb[:, 0:784].rearrange("p (two x) -> p two x", two=2),
                        in_=ps2(ps),
                    )
                    # nu_t = rcp * xnn_s -> bf16 (1e6 cap automatic via eps tap)
                    nc.gpsimd.tensor_mul(
                        i4(uB, b),
                        rcpb[:, 0:784].rearrange(
                            "p (two r c) -> p two r c", two=2, c=28),
                        i4x(xnn, b),
                    )
            def iter16_b(ts):
                xpad, xnn, hT, nuT, hB, uB, yq = ts
                for b in range(NBLK):
                    nu3 = pad3(uB, b)
                    ps = psA.tile([128, 1024], F32, tag="conv")
                    for (r0, c0) in HL:
                        for t in range(9):
                            ky, kx = t // 3, t % 3
                            nc.tensor.matmul(
                                out=ps[:, c0:c0 + N14],
                                lhsT=wfwd[b][:, t, :],
                                rhs=nu3[:, r0 + ky - 1: r0 + ky + 13,
                                        kx: kx + 28],
                                start=(t == 0),
                                stop=(t == 8),
                            )
                    nc.vector.tensor_mul(i4(uB, b), ps4(ps), i4(hB, b))
                # h = u / colsum  (x SH)
                ss = psB.tile([128, 1024], F32, tag="sum")
                for (r0, c0) in HL:
                    for b in range(4):
                        nc.tensor.matmul(
                            out=ss[:, c0:c0 + N14],
                            lhsT=ones_cs,
                            rhs=pad3(uB, b)[:, r0:r0 + 14, 1:29],
                            start=(b == 0),
                            stop=(b == 3),
                        )
                sinvt = stats.tile([128, RL], F32, tag="mstat", bufs=4)
                nc.vector.reciprocal_approx_fast(
                    out=sinvt[:, 0:784].rearrange("p (two x) -> p two x", two=2),
                    in_=ps2(ss),
                )
                sinv4 = sinvt[:, 0:784].rearrange(
                    "p (two r c) -> p two r c", two=2, c=28)
                for b in range(NBLK):
                    nc.gpsimd.tensor_mul(i4(hB, b), sinv4, i4(uB, b))

            def tail_ln(img, ts):
                xpad, xnn, hT, nuT, hB, uB, yq = ts
                # residual: x2 = x + h  (h stored x SH) -> into xnn slot
                for b in range(NBLK):
                    nc.vector.scalar_tensor_tensor(
                        out=xnn[:, b, R0:R0 + RL],
                        in0=hB[:, b, R0:R0 + RL], scalar=1.0 / SH,
                        in1=xpad[:, b, R0:R0 + RL].bitcast(F32),
                        op0=op.mult, op1=op.add,
                    )
                # LN2 -> yq (fp8, feeds the DoubleRow fc1)
                layernorm(
                    xnn,
                    lambda b: yq[:, b, 0:RL],
                    eps2_t, g2t, b2t, "ln2",
                )

            def tail_mlp(img, ts):
                xpad, xnn, hT, nuT, hB, uB, yq = ts
                hid = pimg.tile([128, NJ, RL], FP8, tag="hid", bufs=1,
                                name=f"hid{img}")
                for j in range(NJ):
                    hp = psA.tile([128, 1024], F32, tag="conv")
                    for (c0, cn) in RCH:
                        for p in range(2):
                            nc.tensor.matmul(
                                out=hp[:, c0:c0 + cn],
                                lhsT=w1qt[p][:, j, :, :],
                                rhs=yq[:, 2 * p:2 * p + 2, c0:c0 + cn],
                                start=(p == 0),
                                stop=(p == 1),
                                perf_mode=PM.DoubleRow,
                            )
                    nc.scalar.activation(
                        out=hid[:, j, :], in_=hp[:, 0:RL], func=AF.Gelu,
                        bias=bf1t[:, j:j + 1], scale=1.0 / 32,
                    )
                for cb in range(NBLK):
                    ops_ = psB.tile([128, 1024], F32, tag="sum")
                    for (c0, cn) in RCH:
                        nc.tensor.matmul(
                            out=ops_[:, c0:c0 + cn],
                            lhsT=bf2row[:, cb, :],
                            rhs=ones_row[:, c0:c0 + cn],
                            start=True,
                            stop=False,
                        )
                        for p in range(6):
                            nc.tensor.matmul(
                                out=ops_[:, c0:c0 + cn],
                                lhsT=w2qt[:, p, cb, :, :],
                                rhs=hid[:, 2 * p:2 * p + 2, c0:c0 + cn],
                                start=False,
                                stop=(p == 5),
                                perf_mode=PM.DoubleRow,
                            )
                    nc.vector.scalar_tensor_tensor(
                        out=xpad[:, cb, R0:R0 + RL],
                        in0=ops_[:, 0:RL],
                        scalar=1.0 / 32,
                        in1=xnn[:, cb, R0:R0 + RL].bitcast(F32),
                        op0=op.mult, op1=op.add,
                    )
                for b in range(NBLK):
                    src = (xpad[:, b, :].bitcast(F32)
                           .rearrange("p (r c) -> p r c", c=PW)[:, 1:29, 1:29])
                    nc.sync.dma_start(
                        out=out_ext[img, b * 128:(b + 1) * 128, :, :],
                        in_=src,
                    )

            for p0 in range(0, NB, 2):
                imgs = list(range(p0, min(p0 + 2, NB)))
                tsets = {img: setup_image(img) for img in imgs}
                for it in range(N8):
                    for img in imgs:
                        iter8_a(tsets[img])
                    for img in imgs:
                        iter8_b(tsets[img], last=(it == N8 - 1))
                for it in range(NB16):
                    for img in imgs:
                        iter16_a(tsets[img])
                    for img in imgs:
                        iter16_b(tsets[img])
                for img in imgs:
                    tail_ln(img, tsets[img])
                for img in imgs:
                    tail_mlp(img, tsets[img])

    try:
        nc.compile()
    finally:
        bacc.get_activation_tables = _orig_tables
    return nc


def _prep_weights(Wc, g1, b1, g2, b2, w_fc1, b_fc1, w_fc2, b_fc2):
    import ml_dtypes

    wp = np.abs(np.asarray(Wc, np.float32))
    wp = wp / np.maximum(wp.sum(axis=(1, 2, 3), keepdims=True), EPS)
    wp4 = wp.reshape(NBLK, 2, 64, 64, 3, 3)  # [b, gi, co, ci, ky, kx]
    afwd = np.zeros((NBLK, 128, 9, 128), np.float32)
    abwd = np.zeros((NBLK, 128, 10, 128), np.float32)
    for b in range(NBLK):
        for gi in range(2):
            blk = wp4[b, gi]
            afwd[b, gi * 64:(gi + 1) * 64, :, gi * 64:(gi + 1) * 64] = (
                blk.transpose(1, 2, 3, 0).reshape(64, 9, 64)
            )
            abwd[b, gi * 64:(gi + 1) * 64, 0:9, gi * 64:(gi + 1) * 64] = (
                blk[:, :, ::-1, ::-1].transpose(0, 2, 3, 1).reshape(64, 9, 64)
            )
    abwd[:, 0, 9, :] = EPSWT
    wf8 = np.zeros((NBLK, 128, 10, 128), np.float32)
    wf8[:, :, 0:9, :] = afwd * SWF
    wb8 = abwd * SWB
    wb8[:, :, 9, :] = 0.0
    wb8[:, 0, 9, :] = EPSW8
    wf8 = wf8.astype(ml_dtypes.float8_e4m3)
    wb8 = wb8.astype(ml_dtypes.float8_e4m3)
    assert np.isfinite(wf8.astype(np.float32)).all()
    assert np.isfinite(wb8.astype(np.float32)).all()
    w1r = (np.asarray(w_fc1, np.float32) * 32.0).reshape(NBLK, 128, NJ, 128)
    w1q = np.zeros((2, 128, NJ, 2, 128), np.float32)
    w1q[0, :, :, 0, :] = w1r[0]
    w1q[0, :, :, 1, :] = w1r[1]
    w1q[1, :, :, 0, :] = w1r[2]
    w2r = (np.asarray(w_fc2, np.float32) * 32.0).reshape(NJ, 128, NBLK, 128)
    w2q = np.zeros((128, 6, NBLK, 2, 128), np.float32)
    for p in range(6):
        w2q[:, p, :, 0, :] = w2r[2 * p]
        w2q[:, p, :, 1, :] = w2r[2 * p + 1]
    w1q = w1q.astype(ml_dtypes.float8_e4m3)
    w2q = w2q.astype(ml_dtypes.float8_e4m3)
    assert np.isfinite(w1q.astype(np.float32)).all()
    assert np.isfinite(w2q.astype(np.float32)).all()
    bf2b = (np.asarray(b_fc2, np.float32) * 32.0).reshape(1, NBLK, 128).astype(
        ml_dtypes.bfloat16
    )
    return {
        "afwd": (afwd / 32.0).astype(ml_dtypes.bfloat16),
        "abwd": abwd.astype(ml_dtypes.bfloat16),
        "wf8": wf8,
        "wb8": wb8,
        "w1q": w1q,
        "w2q": w2q,
        "bf2b": bf2b,
        "g1": np.asarray(g1, np.float32).reshape(NBLK, 128),
        "b1": np.asarray(b1, np.float32).reshape(NBLK, 128),
        "g2": np.asarray(g2, np.float32).reshape(NBLK, 128),
        "b2": np.asarray(b2, np.float32).reshape(NBLK, 128),
        "bf1": np.asarray(b_fc1, np.float32).reshape(NJ, 128),
        "bf2": np.asarray(b_fc2, np.float32).reshape(NBLK, 128),
    }


_last_result = None


def kernel(x, g1, b1, Wc, g2, b2, w_fc1, b_fc1, w_fc2, b_fc2):
    global _last_result
    if os.environ.get("JAX_PLATFORMS", "").strip().lower() == "cpu":
        del os.environ["JAX_PLATFORMS"]
    from concourse.bass_utils import run_bass_kernel_spmd

    if "nc" not in _cache:
        _cache["nc"] = _build()
    nc = _cache["nc"]

    shared = _prep_weights(Wc, g1, b1, g2, b2, w_fc1, b_fc1, w_fc2, b_fc2)
    x = np.asarray(x, np.float32)
    assert x.shape == (NB * NCORES, DIM, H, W), x.shape
    in_maps = []
    for c in range(NCORES):
        m = dict(shared)
        m["x"] = np.ascontiguousarray(x[c * NB:(c + 1) * NB])
        in_maps.append(m)

    r = run_bass_kernel_spmd(
        nc, in_maps, list(range(NCORES)),
        trace=bool(os.environ.get("K_TRACE")),
    )
    _last_result = r
    out = np.concatenate(
        [r.results[c]["out"] for c in range(NCORES)], axis=0
    ).astype(np.float32)
    return out
